# revision 1
# baseline (speedup 1.0000x reference)
"""Correlation / cost-volume kernel for Trainium2 (Bass/Tile), 8 NeuronCores.

Problem: out[b, dy*9+dx, y, x] = mean_c in1[b,c,y,x] * pad(in2)[b,c,y+dy,x+dx]
  shapes: in1, in2 [8, 192, 128, 128] f32 -> out [8, 81, 128, 128] f32
  (max_displacement = pad = 4, window 9x9 = 81 displacements)

Distribution: data-parallel over batch; core b handles batch element b.

Per-core algorithm ("Gram row-slab" formulation):
  For each output row y, one matmul group computes
     psi_y[x, (x', dy)] = sum_c in1[c,y,x] * pad(in2)[c, y+dy, x']
  with lhsT = in1 row [C, 128] (C=192 split into K-chunks 128+64) and the
  moving operand streamed from a padded in2 row-slab with column order
  (dy outer within x'-group), N split into 4 PSUM-bank-sized matmuls of
  306 columns (34 x'-groups x 9 dy) in float32r (full-rate fp32 path).
  The 81 outputs for pixel (y, x) are then the contiguous run
  psi_y[x, 9x : 9x+81] (dx outer, dy inner) -- extraction of the
  band-diagonal reduces to per-16-partition-block staircase windows,
  which are DMA'd to a DRAM staging tensor; the final pure-indexing
  gather to [81, H, W] happens on the host (no arithmetic).

  in1 is pre-scaled by 1/C on the host so no on-device scaling is needed.
"""
import sys

sys.path.insert(0, "/opt/trn_rl_repo")

import numpy as np

_RUNNER_CACHE = {}

# problem constants (hardcoded per harness contract)
B, C, H, W, MAXD = 8, 192, 128, 128, 4
WIN = 2 * MAXD + 1  # 9
XP = W + 2 * MAXD  # 136 padded x'
GPB = 34  # x'-groups per PSUM bank-matmul
NB = 4  # N-splits (banks) per y
BLK = 16  # partition block for staircase windows
NBLK = W // BLK  # 8
WINX = BLK + 2 * MAXD  # 24 x'-window per block
NYB = 8  # y rows batched per stage DMA group
TY = 16  # y-tile


def _build(nc):
    import concourse.mybir as mybir
    from concourse.tile import TileContext

    F32 = mybir.dt.float32
    F32R = mybir.dt.float32r

    in1 = nc.declare_dram_parameter("in1", [C, H, W], F32, isOutput=False)
    in2 = nc.declare_dram_parameter("in2", [C, H, W], F32, isOutput=False)
    stage = nc.declare_dram_parameter(
        "stage", [NBLK, BLK, H, WINX, WIN], F32, isOutput=True
    )
    NR = TY + 2 * MAXD
    ntiles = H // TY

    with TileContext(nc) as tc:
        with (
            tc.tile_pool(name="w", bufs=2) as wpool,
            tc.tile_pool(name="wn", bufs=1) as wnpool,
            tc.tile_pool(name="a", bufs=2) as apool,
            tc.tile_pool(name="s", bufs=2) as spool,
            tc.tile_pool(name="psum", bufs=2, space="PSUM") as ppool,
        ):
            for t in range(ntiles):
                Y0 = t * TY
                nr = TY + 2 * MAXD
                r_lo = max(0, 4 - Y0)
                r_hi = min(nr, H + 4 - Y0)

                # fp32 natural-layout in2 window [c, y'slot, x'] via Pool SWDGE
                wn1 = wnpool.tile([128, NR, XP], F32, tag="wn1")
                wn2 = wnpool.tile([64, NR, XP], F32, tag="wn2")
                for wn, c0, cn in ((wn1, 0, 128), (wn2, 128, 64)):
                    nc.gpsimd.memset(wn[:cn, :, 0:MAXD], 0.0)
                    nc.gpsimd.memset(wn[:cn, :, MAXD + W : XP], 0.0)
                    if r_lo > 0:
                        nc.gpsimd.memset(wn[:cn, 0:r_lo, :], 0.0)
                    if r_hi < nr:
                        nc.gpsimd.memset(wn[:cn, r_hi:nr, :], 0.0)
                    nc.gpsimd.dma_start(
                        out=wn[:cn, r_lo:r_hi, MAXD : MAXD + W],
                        in_=in2[c0 : c0 + cn, Y0 - 4 + r_lo : Y0 - 4 + r_hi, :],
                    )
                # repack to [c, x', y'] (y' contiguous -- float32r needs a
                # stride-1 outermost free dim on the moving operand), with
                # fp32 -> f32r rounding.  chunk1 on Pool, chunk2 split DVE/ACT.
                wt1 = wpool.tile([128, XP, NR], F32R, tag="wt1")
                wt2 = wpool.tile([64, XP, NR], F32R, tag="wt2")
                nc.gpsimd.tensor_copy(wt1[:, :, :], wn1[:, :, :].transpose([0, 2, 1]))
                hx = XP // 2
                nc.vector.tensor_copy(
                    wt2[:64, 0:hx, :], wn2[:64, :, 0:hx].transpose([0, 2, 1])
                )
                nc.scalar.copy(
                    wt2[:64, hx:XP, :], wn2[:64, :, hx:XP].transpose([0, 2, 1])
                )

                # in1 rows cast to f32r (values pre-scaled by 1/C on host)
                a1 = apool.tile([128, TY, W], F32R, tag="a1")
                a2 = apool.tile([64, TY, W], F32R, tag="a2")
                nc.gpsimd.dma_start(out=a1[:, :, :], in_=in1[0:128, Y0 : Y0 + TY, :])
                nc.gpsimd.dma_start(
                    out=a2[:64, :, :], in_=in1[128:192, Y0 : Y0 + TY, :]
                )

                for g in range(TY // NYB):
                    s4 = spool.tile([128, NYB, XP, WIN], F32, tag="s4")
                    for k in range(NYB):
                        yy = g * NYB + k
                        psi_lo = ppool.tile([128, 2 * 512], F32, tag="psi_lo")
                        psi_hi = ppool.tile([128, 2 * 512], F32, tag="psi_hi")
                        for b in range(NB):
                            psi = psi_lo if b < 2 else psi_hi
                            bb = b % 2
                            for ci, (wt, at, cn) in enumerate(
                                ((wt1, a1, 128), (wt2, a2, 64))
                            ):
                                rhs = wt[
                                    :cn, GPB * b : GPB * (b + 1), yy : yy + WIN
                                ].transpose([0, 2, 1])
                                nc.tensor.matmul(
                                    psi[:, 512 * bb : 512 * bb + GPB * WIN],
                                    at[:cn, yy, :],
                                    rhs,
                                    start=(ci == 0),
                                    stop=(ci == 1),
                                )
                        # evict PSUM -> s4 slot: DVE lo half, ACT hi half
                        sv = s4[:, k, :, :].rearrange("p (b g) d -> p b d g", b=NB)
                        for eng, psi, b0 in (
                            (nc.vector, psi_lo, 0),
                            (nc.scalar, psi_hi, 2),
                        ):
                            src = (
                                psi[:, :]
                                .rearrange("p (b r) -> p b r", b=2)[
                                    :, :, 0 : GPB * WIN
                                ]
                                .rearrange("p b (d g) -> p b d g", d=WIN)
                            )
                            dst = sv[:, b0 : b0 + 2]
                            if eng is nc.vector:
                                nc.vector.tensor_copy(dst, src)
                            else:
                                nc.scalar.copy(dst, src)
                    # staircase-window stage DMAs (HWDGE/SP)
                    for blk in range(NBLK):
                        dst = stage[
                            blk, :, Y0 + g * NYB : Y0 + g * NYB + NYB, :, :
                        ].rearrange("p y w d -> p y (w d)")
                        nc.sync.dma_start(
                            out=dst,
                            in_=s4[
                                BLK * blk : BLK * (blk + 1),
                                :,
                                BLK * blk : BLK * blk + WINX,
                                :,
                            ].rearrange("p y w d -> p y (w d)"),
                        )
    return stage


def _get_runner():
    if "r" in _RUNNER_CACHE:
        return _RUNNER_CACHE["r"]
    import concourse.bacc as bacc
    from concourse.bass_utils import run_bass_kernel_spmd

    nc = bacc.Bacc("TRN2", target_bir_lowering=False, debug=False, num_devices=B)
    _build(nc)
    nc.compile()

    def run(in_maps):
        return run_bass_kernel_spmd(nc, in_maps, list(range(B)))

    _RUNNER_CACHE["r"] = run
    return run


def _host_gather(stage_v):
    """stage [NBLK, BLK(pp), H, WINX(xw), WIN(dy)] -> out [81, H, W].

    out[dy*9+dx, y, 16*blk+pp] = stage[blk, pp, y, pp+dx, dy]
    (pure indexing -- all arithmetic was done on device)
    """
    out = np.empty((WIN * WIN, H, W), dtype=np.float32)
    for pp in range(BLK):
        sl = stage_v[:, pp, :, pp : pp + WIN, :]  # [blk, y, dx, dy]
        out[:, :, pp::BLK] = sl.transpose(3, 2, 1, 0).reshape(WIN * WIN, H, NBLK)
    return out


def kernel(in1, in2):
    in1 = np.ascontiguousarray(np.asarray(in1, dtype=np.float32))
    in2 = np.ascontiguousarray(np.asarray(in2, dtype=np.float32))
    assert in1.shape == (B, C, H, W) and in2.shape == (B, C, H, W)
    run = _get_runner()
    scale = np.float32(1.0 / C)
    in_maps = [
        {"in1": in1[b] * scale, "in2": in2[b]} for b in range(B)
    ]
    res = run(in_maps)
    out = np.empty((B, WIN * WIN, H, W), dtype=np.float32)
    for b in range(B):
        out[b] = _host_gather(res.results[b]["stage"])
    return out
